# revision 4
# baseline (speedup 1.0000x reference)
"""HNLoRALinear Trainium2 kernel.

out[b,s,o] = x[b] @ W^T + bias + SCALE * (x[b] @ A[b]) @ B[b]

Sharding: 8 cores = 4 batches x 2 sequence-halves. Each core computes
its [1024 tokens, 4096 outs] output block, TRANSPOSED on device
(outs on PSUM partitions, tokens as the moving dim) so that:
  - the stationary matmul operand is a [128, 128] W^T chunk,
  - the moving operand is a 512-token slice of the SBUF-resident x^T
    (N=512 stream time ~213ns fully hides the ~197ns 4-byte f32r
    weight load; at N=256 the load was exposed),
  - consecutive k-chunk matmuls accumulate in PSUM.
The host pre-transposes x and W so the contraction dim (D_IN) lands on
SBUF partitions with no on-device transposes, and un-transposes each
core's [4096, 1024] output block when assembling the result.

The per-sample LoRA correction + bias ride along as one extra K=17
matmul per output tile: [SCALE*B ; bias]^T-chunk (stationary) @
[low ; ones] (moving), accumulated into the same PSUM group.

Matmuls run in float32r (TF32-like, full PE rate at moving-dim>=256,
~1.2e-4 rel err end-to-end).
"""
import numpy as np

import concourse.bass as bass  # noqa: F401  (bass must import before tile)
import concourse.mybir as mybir
import concourse.tile as tile
from concourse import bacc
from concourse.bass_utils import run_bass_kernel_spmd

# Problem shapes (hardcoded per contract).
B, S, D_IN, D_OUT, R = 4, 2048, 4096, 4096, 16
SCALE = 32.0 / 16.0
SH = S // 2            # tokens per core
P = 128
KC = D_IN // P         # 32 contraction chunks
O_CHUNKS = D_OUT // P  # 32 output-feature chunks (PSUM partition dim)
TN = 512               # moving-dim token group width
TGROUPS = SH // TN     # 2
RA = R + 1             # augmented rank (lora + bias row)

_cached_nc = None


def _build():
    f32r = mybir.dt.float32r
    f32 = mybir.dt.float32
    nc = bacc.Bacc(
        "TRN2", target_bir_lowering=False, debug=False, enable_asserts=False
    )
    xt = nc.dram_tensor("xt", [D_IN, SH], f32r, kind="ExternalInput")
    wt = nc.dram_tensor("wt", [D_IN, D_OUT], f32r, kind="ExternalInput")
    apk = nc.dram_tensor("apack", [P, KC * R], f32r, kind="ExternalInput")
    bga = nc.dram_tensor("baug", [RA, D_OUT], f32r, kind="ExternalInput")
    ot_d = nc.dram_tensor("ot", [D_OUT, SH], f32, kind="ExternalOutput")

    with tile.TileContext(nc) as tc:
        with (
            tc.tile_pool(name="xp", bufs=KC) as xp,
            tc.tile_pool(name="wp", bufs=2 * KC) as wp,
            tc.tile_pool(name="cp", bufs=1) as cp,
            tc.tile_pool(name="op", bufs=4) as op,
            tc.tile_pool(name="pp", bufs=4, space="PSUM") as pp,
            tc.tile_pool(name="lp", bufs=2, space="PSUM") as lp,
        ):
            at = cp.tile([P, KC * R], f32r, name="at")
            nc.sync.dma_start(out=at[:], in_=apk.ap())
            bt = cp.tile([RA, D_OUT], f32r, name="bt")
            nc.sync.dma_start(out=bt[:], in_=bga.ap())

            # Interleave x chunks with W strip 0 so the main pipeline can
            # start as soon as the first chunks land.
            def load_w_tile(o, k):
                wk = wp.tile([P, P], f32r, name="wk")
                nc.sync.dma_start(
                    out=wk[:],
                    in_=wt.ap()[k * P : (k + 1) * P, o * P : (o + 1) * P],
                )
                return wk

            xts = []
            w_strip = []
            for k in range(KC):
                xk = xp.tile([P, SH], f32r, name="xk")
                nc.sync.dma_start(out=xk[:], in_=xt.ap()[k * P : (k + 1) * P, :])
                xts.append(xk)
                w_strip.append(load_w_tile(0, k))

            # Augmented low-rank activations: rows 0..15 = (x @ A)^T,
            # row 16 = 1. (memset must start at a 32-aligned partition, so
            # fill all 17 rows; the copies below overwrite rows 0..15.)
            low = cp.tile([RA, SH], f32r, name="low")
            nc.gpsimd.memset(low[:].bitcast(f32), 1.0)
            pls = [lp.tile([R, TN], f32, name="pl") for _ in range(TGROUPS)]
            for k in range(KC):
                for t in range(TGROUPS):
                    nc.tensor.matmul(
                        pls[t][:],
                        at[:, k * R : (k + 1) * R],
                        xts[k][:, t * TN : (t + 1) * TN],
                        start=(k == 0),
                        stop=(k == KC - 1),
                    )
            for t in range(TGROUPS):
                nc.vector.tensor_copy(low[0:R, t * TN : (t + 1) * TN], pls[t][:])

            for o in range(O_CHUNKS):
                if o > 0:
                    w_strip = [load_w_tile(o, k) for k in range(KC)]
                for t in range(TGROUPS):
                    ps = pp.tile([P, TN], f32, name="ps")
                    for k in range(KC):
                        nc.tensor.matmul(
                            ps[:],
                            w_strip[k][:],
                            xts[k][:, t * TN : (t + 1) * TN],
                            start=(k == 0),
                            stop=False,
                        )
                    nc.tensor.matmul(
                        ps[:],
                        bt[:, o * P : (o + 1) * P],
                        low[:, t * TN : (t + 1) * TN],
                        start=False,
                        stop=True,
                    )
                    otile = op.tile([P, TN], f32, name="otile")
                    nc.vector.tensor_copy(otile[:], ps[:])
                    nc.sync.dma_start(
                        out=ot_d.ap()[o * P : (o + 1) * P, t * TN : (t + 1) * TN],
                        in_=otile[:],
                    )
    nc.compile()
    return nc


def _get_nc():
    global _cached_nc
    if _cached_nc is None:
        _cached_nc = _build()
    return _cached_nc


def _in_maps(x, weight, bias, lora_A, lora_B):
    wt = np.ascontiguousarray(weight.T).astype(np.float32, copy=False)
    bias = bias.astype(np.float32, copy=False)
    maps = []
    for c in range(8):
        b, h = divmod(c, 2)
        xtc = np.ascontiguousarray(x[b, h * SH : (h + 1) * SH, :].T).astype(
            np.float32, copy=False
        )
        apk = np.ascontiguousarray(
            lora_A[b].reshape(KC, P, R).transpose(1, 0, 2).reshape(P, KC * R)
        ).astype(np.float32, copy=False)
        baug = np.concatenate(
            [lora_B[b].astype(np.float32) * np.float32(SCALE), bias[None, :]], axis=0
        )
        maps.append({"xt": xtc, "wt": wt, "apack": apk, "baug": baug})
    return maps


def kernel(x, weight, bias, lora_A, lora_B, _trace=False, _tmpdir=None):
    x = np.asarray(x, dtype=np.float32)
    weight = np.asarray(weight, dtype=np.float32)
    bias = np.asarray(bias, dtype=np.float32)
    lora_A = np.asarray(lora_A, dtype=np.float32)
    lora_B = np.asarray(lora_B, dtype=np.float32)

    nc = _get_nc()
    maps = _in_maps(x, weight, bias, lora_A, lora_B)
    res = run_bass_kernel_spmd(
        nc, maps, list(range(8)), trace=_trace, tmpdir=_tmpdir
    )
    out = np.empty((B, S, D_OUT), np.float32)
    for c in range(8):
        b, h = divmod(c, 2)
        out[b, h * SH : (h + 1) * SH, :] = res.results[c]["ot"].T
    if _trace:
        return out, res
    return out


# revision 6
# speedup vs baseline: 1.4332x; 1.4332x over previous
"""HNLoRALinear Trainium2 kernel.

out[b,s,o] = x[b] @ W^T + bias + SCALE * (x[b] @ A[b]) @ B[b]

Sharding: 8 cores = 4 batches x 2 sequence-halves. Each core computes
its [1024 tokens, 4096 outs] output block, TRANSPOSED on device
(outs on PSUM partitions, tokens as the moving dim) so that:
  - the stationary matmul operand is a [128, 128] W^T chunk,
  - the moving operand is a 512-token slice of the SBUF-resident x^T
    (N=512 stream time ~213ns fully hides the ~197ns 4-byte f32r
    weight load; at N=256 the load was exposed),
  - consecutive k-chunk matmuls accumulate in PSUM.
The host pre-transposes x and W so the contraction dim (D_IN) lands on
SBUF partitions with no on-device transposes, and un-transposes each
core's [4096, 1024] output block when assembling the result.

The per-sample LoRA correction + bias ride along as one extra K=17
matmul per output tile: [SCALE*B ; bias]^T-chunk (stationary) @
[low ; ones] (moving), accumulated into the same PSUM group.

Matmuls run in float32r (TF32-like, full PE rate at moving-dim>=256,
~1.2e-4 rel err end-to-end).
"""
import numpy as np

import concourse.bass as bass  # noqa: F401  (bass must import before tile)
import concourse.mybir as mybir
import concourse.tile as tile
from concourse import bacc
from concourse.bass_utils import run_bass_kernel_spmd

# Problem shapes (hardcoded per contract).
B, S, D_IN, D_OUT, R = 4, 2048, 4096, 4096, 16
SCALE = 32.0 / 16.0
SH = S // 2            # tokens per core
P = 128
KC = D_IN // P         # 32 contraction chunks
O_CHUNKS = D_OUT // P  # 32 output-feature chunks (PSUM partition dim)
TN = 512               # moving-dim token group width
TGROUPS = SH // TN     # 2
RA = R + 1             # augmented rank (lora + bias row)

_cached_nc = None


def _build():
    f32r = mybir.dt.float32r
    f32 = mybir.dt.float32
    nc = bacc.Bacc(
        "TRN2", target_bir_lowering=False, debug=False, enable_asserts=False
    )
    xt = nc.dram_tensor("xt", [D_IN, SH], f32r, kind="ExternalInput")
    wt = nc.dram_tensor("wt", [D_IN, D_OUT], f32r, kind="ExternalInput")
    apk = nc.dram_tensor("apack", [P, KC * R], f32r, kind="ExternalInput")
    bga = nc.dram_tensor("baug", [RA, D_OUT], f32r, kind="ExternalInput")
    ot_d = nc.dram_tensor("ot", [D_OUT, SH], f32, kind="ExternalOutput")

    # 3D views putting the within-chunk row on partitions: element
    # (k*128+p, col) -> [p, k, col].
    xt3 = xt.ap().rearrange("(k p) s -> p k s", p=P)
    wt3 = wt.ap().rearrange("(k p) o -> p k o", p=P)
    XG = 4  # x DMA groups
    KG = KC // XG

    with tile.TileContext(nc) as tc:
        with (
            tc.tile_pool(name="xp", bufs=1) as xp,
            tc.tile_pool(name="wp", bufs=2) as wp,
            tc.tile_pool(name="cp", bufs=1) as cp,
            tc.tile_pool(name="op", bufs=3) as op,
            tc.tile_pool(name="pp", bufs=4, space="PSUM") as pp,
            tc.tile_pool(name="lp", bufs=2, space="PSUM") as lp,
        ):
            at = cp.tile([P, KC * R], f32r, name="at")
            nc.sync.dma_start(out=at[:], in_=apk.ap())
            bt = cp.tile([RA, D_OUT], f32r, name="bt")
            nc.sync.dma_start(out=bt[:], in_=bga.ap())

            def load_w_strip(o):
                wk = wp.tile([P, KC, P], f32r, name="wk")
                nc.sync.dma_start(out=wk[:], in_=wt3[:, :, o * P : (o + 1) * P])
                return wk

            # x^T fully resident as one [128, 32, 1024] tile, loaded in 4
            # chunk-groups interleaved with the first W strips so the
            # low-rank + first main matmuls can start early.
            xr = xp.tile([P, KC, SH], f32r, name="xr")
            w_strips = {}
            for g in range(XG):
                nc.sync.dma_start(
                    out=xr[:, g * KG : (g + 1) * KG, :],
                    in_=xt3[:, g * KG : (g + 1) * KG, :],
                )
                if g < 2:
                    w_strips[g] = load_w_strip(g)

            # Augmented low-rank activations: rows 0..15 = (x @ A)^T,
            # row 16 = 1. (memset must start at a 32-aligned partition, so
            # fill all 17 rows; the copies below overwrite rows 0..15.)
            low = cp.tile([RA, SH], f32r, name="low")
            nc.gpsimd.memset(low[:].bitcast(f32), 1.0)
            pls = [lp.tile([R, TN], f32, name="pl") for _ in range(TGROUPS)]
            for k in range(KC):
                for t in range(TGROUPS):
                    nc.tensor.matmul(
                        pls[t][:],
                        at[:, k * R : (k + 1) * R],
                        xr[:, k, t * TN : (t + 1) * TN],
                        start=(k == 0),
                        stop=(k == KC - 1),
                    )
            for t in range(TGROUPS):
                nc.vector.tensor_copy(low[0:R, t * TN : (t + 1) * TN], pls[t][:])

            for o in range(O_CHUNKS):
                wk = w_strips.pop(o) if o in w_strips else load_w_strip(o)
                if o + 1 < O_CHUNKS and o >= 1 and (o + 1) not in w_strips:
                    w_strips[o + 1] = load_w_strip(o + 1)
                otile = op.tile([P, SH], f32, name="otile")
                for t in range(TGROUPS):
                    ps = pp.tile([P, TN], f32, name="ps")
                    for k in range(KC):
                        nc.tensor.matmul(
                            ps[:],
                            wk[:, k, :],
                            xr[:, k, t * TN : (t + 1) * TN],
                            start=(k == 0),
                            stop=False,
                        )
                    nc.tensor.matmul(
                        ps[:],
                        bt[:, o * P : (o + 1) * P],
                        low[:, t * TN : (t + 1) * TN],
                        start=False,
                        stop=True,
                    )
                    nc.vector.tensor_copy(otile[:, t * TN : (t + 1) * TN], ps[:])
                nc.sync.dma_start(out=ot_d.ap()[o * P : (o + 1) * P, :], in_=otile[:])
    nc.compile()
    return nc


def _get_nc():
    global _cached_nc
    if _cached_nc is None:
        _cached_nc = _build()
    return _cached_nc


def _in_maps(x, weight, bias, lora_A, lora_B):
    wt = np.ascontiguousarray(weight.T).astype(np.float32, copy=False)
    bias = bias.astype(np.float32, copy=False)
    maps = []
    for c in range(8):
        b, h = divmod(c, 2)
        xtc = np.ascontiguousarray(x[b, h * SH : (h + 1) * SH, :].T).astype(
            np.float32, copy=False
        )
        apk = np.ascontiguousarray(
            lora_A[b].reshape(KC, P, R).transpose(1, 0, 2).reshape(P, KC * R)
        ).astype(np.float32, copy=False)
        baug = np.concatenate(
            [lora_B[b].astype(np.float32) * np.float32(SCALE), bias[None, :]], axis=0
        )
        maps.append({"xt": xtc, "wt": wt, "apack": apk, "baug": baug})
    return maps


def kernel(x, weight, bias, lora_A, lora_B, _trace=False, _tmpdir=None):
    x = np.asarray(x, dtype=np.float32)
    weight = np.asarray(weight, dtype=np.float32)
    bias = np.asarray(bias, dtype=np.float32)
    lora_A = np.asarray(lora_A, dtype=np.float32)
    lora_B = np.asarray(lora_B, dtype=np.float32)

    nc = _get_nc()
    maps = _in_maps(x, weight, bias, lora_A, lora_B)
    res = run_bass_kernel_spmd(
        nc, maps, list(range(8)), trace=_trace, tmpdir=_tmpdir
    )
    out = np.empty((B, S, D_OUT), np.float32)
    for c in range(8):
        b, h = divmod(c, 2)
        out[b, h * SH : (h + 1) * SH, :] = res.results[c]["ot"].T
    if _trace:
        return out, res
    return out
